# revision 1
# baseline (speedup 1.0000x reference)
"""Trainium2 Bass kernel for CausalSelfAttention with sliding-window + sink mask.

Sharding: 8 cores = (batch 2) x (sequence chunks of 512). Each core computes
QKV (+RoPE) for its 512 queries and for a kv range [4 sink | 256 halo |
512 own | 124 zero-pad] = 896 positions, runs banded attention in a
scores-transposed [k, q] layout (7 key-chunks of 128 with fixed q-windows,
multiplicative post-exp masking, denominator via a ones-column in V), then
projects with w_proj emitting a transposed [C, 512] output that the host
re-transposes and concatenates.

All matmuls run as float32r (full-rate fp32 path of the PE).
"""

import numpy as np

B, T, C, NH, HD = 2, 2048, 1024, 16, 64
WIN, SINK = 256, 4
CH = 512          # queries per core
KV = 896          # 512 own + 256 halo + 4 sink + 124 pad
NCORES = 8
W_C = [384, 512, 256, 256, 128, 256, 512]
OFF_C = [0, 0, 256, 256, 0, 0, 0]
MOFF = np.concatenate([[0], np.cumsum(W_C)]).astype(int)  # mask col offsets
MTOT = int(MOFF[-1])  # 1924

_cache = {}


def _build_nc():
    import concourse.bacc as bacc
    import concourse.mybir as mybir
    import concourse.tile as tile

    f32 = mybir.dt.float32
    f32r = mybir.dt.float32r
    AF = mybir.ActivationFunctionType

    nc = bacc.Bacc("TRN2", target_bir_lowering=False, debug=False,
                   num_devices=NCORES)

    xT = nc.dram_tensor("xT", [C, KV], f32r, kind="ExternalInput").ap()
    wqs = nc.dram_tensor("wqs", [C, C], f32r, kind="ExternalInput").ap()
    wks = nc.dram_tensor("wks", [C, C], f32r, kind="ExternalInput").ap()
    wv = nc.dram_tensor("wv", [C, C], f32r, kind="ExternalInput").ap()
    wps = nc.dram_tensor("wps", [C, C], f32r, kind="ExternalInput").ap()
    cos_q = nc.dram_tensor("cos_q", [128, CH], f32, kind="ExternalInput").ap()
    sin_q = nc.dram_tensor("sin_q", [128, CH], f32, kind="ExternalInput").ap()
    cos_k = nc.dram_tensor("cos_k", [128, KV], f32, kind="ExternalInput").ap()
    sin_k = nc.dram_tensor("sin_k", [128, KV], f32, kind="ExternalInput").ap()
    masks = nc.dram_tensor("masks", [128, MTOT], f32, kind="ExternalInput").ap()
    p2d = nc.dram_tensor("p2", [128, 128], f32r, kind="ExternalInput").ap()
    rseld = nc.dram_tensor("rsel", [16, C], f32r, kind="ExternalInput").ap()
    onesd = nc.dram_tensor("ones", [128, 16], f32, kind="ExternalInput").ap()
    outT = nc.dram_tensor("outT", [C, CH], f32, kind="ExternalOutput").ap()

    KSEG = [(0, 512), (512, 384)]  # kv free-dim segments (psum bank limit)

    with tile.TileContext(nc) as tc:
        with (
            tc.tile_pool(name="pers", bufs=1) as pers,
            tc.tile_pool(name="wsl", bufs=2) as wsl,
            tc.tile_pool(name="big", bufs=8) as big,     # wv chunks then praw/outT
            tc.tile_pool(name="qk", bufs=2) as qkp,
            tc.tile_pool(name="tmp", bufs=2) as tmp,
            tc.tile_pool(name="yts", bufs=1) as ytsp,
            tc.tile_pool(name="ptp", bufs=14) as ptp,
            tc.tile_pool(name="sm", bufs=2) as smp,
            tc.tile_pool(name="psmm", bufs=2, space="PSUM") as psmm,
            tc.tile_pool(name="pssc", bufs=4, space="PSUM") as pssc,
            tc.tile_pool(name="psyt", bufs=2, space="PSUM") as psyt,
        ):
            # ---------- persistent loads ----------
            xa, xb = [], []
            for i in range(8):
                t = pers.tile([128, 512], f32r, tag=f"xa{i}", name=f"xa{i}")
                nc.sync.dma_start(t[:], xT[i * 128:(i + 1) * 128, 0:512])
                xa.append(t)
                t = pers.tile([128, 384], f32r, tag=f"xb{i}", name=f"xb{i}")
                nc.sync.dma_start(t[:], xT[i * 128:(i + 1) * 128, 512:896])
                xb.append(t)
            tcos_q = pers.tile([128, CH], f32, tag="cos_q")
            nc.sync.dma_start(tcos_q[:], cos_q[:])
            tsin_q = pers.tile([128, CH], f32, tag="sin_q")
            nc.sync.dma_start(tsin_q[:], sin_q[:])
            tcos_k = pers.tile([128, KV], f32, tag="cos_k")
            nc.sync.dma_start(tcos_k[:], cos_k[:])
            tsin_k = pers.tile([128, KV], f32, tag="sin_k")
            nc.sync.dma_start(tsin_k[:], sin_k[:])
            tmask = pers.tile([128, MTOT], f32, tag="mask")
            nc.sync.dma_start(tmask[:], masks[:])
            tp2 = pers.tile([128, 128], f32r, tag="p2")
            nc.sync.dma_start(tp2[:], p2d[:])
            trsel = pers.tile([16, C], f32r, tag="rsel")
            nc.sync.dma_start(trsel[:], rseld[:])
            tones = pers.tile([128, 16], f32, tag="ones")
            nc.sync.dma_start(tones[:], onesd[:])

            # ---------- V = xT.T @ wv in [k, d] layout with ones columns ----------
            wvc = []
            for kc in range(8):
                t = big.tile([128, 1024], f32r, tag="big", name=f"wvc{kc}")
                nc.sync.dma_start(t[:], wv[kc * 128:(kc + 1) * 128, :])
                wvc.append(t)
            v_sb = []
            for tt in range(7):
                vt = pers.tile([128, 1040], f32r, tag=f"v{tt}", name=f"v{tt}")
                vr = vt.rearrange("p (h e) -> p h e", e=65)
                pv = [psmm.tile([128, 512], f32, tag="mm", name=f"pv{tt}_{i}")
                      for i in range(2)]
                for kc in range(8):
                    xsl = (xa[kc][:, tt * 128:(tt + 1) * 128] if tt < 4 else
                           xb[kc][:, (tt - 4) * 128:(tt - 3) * 128])
                    for dh in range(2):
                        nc.tensor.matmul(
                            pv[dh][:], xsl,
                            wvc[kc][:, dh * 512:(dh + 1) * 512],
                            start=(kc == 0), stop=(kc == 7),
                        )
                for dh in range(2):
                    nc.scalar.copy(
                        vr[:, dh * 8:(dh + 1) * 8, 0:64],
                        pv[dh][:].rearrange("p (h e) -> p h e", e=64),
                    )
                nc.scalar.copy(vr[:, :, 64:65],
                               tones[:].rearrange("p (h o) -> p h o", o=1))
                v_sb.append(vt)

            d16 = smp.tile([16, 512], f32, tag="d16")
            AVORD = [1, 6, 0, 5, 4, 2, 3]

            def qkv_rope(hp):
                # qT raw
                wq_sl = wsl.tile([128, 1024], f32r, tag="wslab",
                                 name=f"wq{hp}")
                nc.sync.dma_start(wq_sl[:], wqs[hp * 128:(hp + 1) * 128, :])
                pq = psmm.tile([128, 512], f32, tag="mm", name=f"pq{hp}")
                for kc in range(8):
                    nc.tensor.matmul(
                        pq[:], wq_sl[:, kc * 128:(kc + 1) * 128],
                        xa[kc][:],
                        start=(kc == 0), stop=(kc == 7),
                    )
                qraw = tmp.tile([128, CH], f32r, tag="qraw", name=f"qraw{hp}")
                nc.scalar.copy(qraw[:], pq[:])

                # kT raw (segments share each weight load)
                wk_sl = wsl.tile([128, 1024], f32r, tag="wslab",
                                 name=f"wk{hp}")
                nc.sync.dma_start(wk_sl[:], wks[hp * 128:(hp + 1) * 128, :])
                kraw = tmp.tile([128, KV], f32r, tag="kraw", name=f"kraw{hp}")
                pk = [psmm.tile([128, 512], f32, tag="mm", name=f"pk{hp}_{i}")
                      for i in range(2)]
                for kc in range(8):
                    for si, (s0, sw) in enumerate(KSEG):
                        rhs = xa[kc][:] if si == 0 else xb[kc][:]
                        nc.tensor.matmul(
                            pk[si][:, 0:sw], wk_sl[:, kc * 128:(kc + 1) * 128],
                            rhs, start=(kc == 0), stop=(kc == 7),
                        )
                for si, (s0, sw) in enumerate(KSEG):
                    nc.scalar.copy(kraw[:, s0:s0 + sw], pk[si][:, 0:sw])

                # rope
                qT = qkp.tile([128, CH], f32r, tag="qT", name=f"qT{hp}")
                prot = psmm.tile([128, 512], f32, tag="mm", name=f"prot{hp}")
                nc.tensor.matmul(prot[:], tp2[:], qraw[:], start=True, stop=True)
                t2 = tmp.tile([128, CH], f32, tag="t2", name=f"t2q{hp}")
                nc.vector.tensor_mul(t2[:], prot[:], tsin_q[:])
                nc.vector.tensor_mul(qraw[:], qraw[:], tcos_q[:])
                nc.vector.tensor_add(qT[:], qraw[:], t2[:])

                kT = qkp.tile([128, KV], f32r, tag="kT", name=f"kT{hp}")
                for si, (s0, sw) in enumerate(KSEG):
                    prk = psmm.tile([128, 512], f32, tag="mm",
                                    name=f"prk{hp}_{si}")
                    nc.tensor.matmul(prk[:, 0:sw], tp2[:],
                                     kraw[:, s0:s0 + sw], start=True, stop=True)
                    t2k = tmp.tile([128, 512], f32, tag="t2",
                                   name=f"t2k{hp}_{si}")
                    nc.vector.tensor_mul(t2k[:, 0:sw], prk[:, 0:sw],
                                         tsin_k[:, s0:s0 + sw])
                    nc.vector.tensor_mul(kraw[:, s0:s0 + sw],
                                         kraw[:, s0:s0 + sw],
                                         tcos_k[:, s0:s0 + sw])
                    nc.vector.tensor_add(kT[:, s0:s0 + sw],
                                         kraw[:, s0:s0 + sw], t2k[:, 0:sw])
                return qT, kT

            def sc_block(hp, qT, kT):
                # scoresT matmuls issued as adjacent row-tile pairs (K=64 at
                # partition bases 0/64 -> concurrent in the PE array), then
                # exp (psum->sbuf, fused 1/sqrt(hd) scale) and mask multiply.
                pts = {}
                for c in range(7):
                    w, off = W_C[c], OFF_C[c]
                    scs = []
                    for half in range(2):
                        dsl = slice(half * 64, half * 64 + 64)
                        sc = pssc.tile([128, 512], f32, tag="sc",
                                       name=f"sc{hp}_{c}_{half}")
                        nc.tensor.matmul(
                            sc[:, 0:w], kT[dsl, c * 128:(c + 1) * 128],
                            qT[dsl, off:off + w], start=True, stop=True,
                        )
                        scs.append(sc)
                    for half in range(2):
                        w, off = W_C[c], OFF_C[c]
                        praw = big.tile([128, 512], f32r, tag="big",
                                        name=f"praw{hp}_{c}_{half}")
                        nc.scalar.activation(praw[:, 0:w], scs[half][:, 0:w],
                                             AF.Exp, scale=0.125)
                        pt = ptp.tile([128, 512], f32r, tag="pt",
                                      name=f"pt{hp}_{c}_{half}")
                        nc.vector.tensor_mul(
                            pt[:, 0:w], praw[:, 0:w],
                            tmask[:, MOFF[c]:MOFF[c] + w],
                        )
                        pts[(c, half)] = pt
                return pts

            def av_block(hp, pts):
                yt_pair = []
                for half in range(2):
                    h = hp * 2 + half
                    yt = psyt.tile([65, 512], f32, tag="yt",
                                   name=f"yt{hp}_{half}")
                    for ci, c in enumerate(AVORD):
                        w, off = W_C[c], OFF_C[c]
                        nc.tensor.matmul(
                            yt[:, off:off + w],
                            v_sb[c][:, h * 65:(h + 1) * 65],
                            pts[(c, half)][:, 0:w],
                            start=(ci == 0), stop=(ci == 6),
                        )
                    yt_pair.append(yt)
                ytu = ytsp.tile([128, CH], f32r, tag=f"ytu{hp}",
                                name=f"ytu{hp}")
                nc.scalar.copy(ytu[0:64, :], yt_pair[0][0:64, :])
                nc.scalar.copy(ytu[64:128, :], yt_pair[1][0:64, :])
                for half in range(2):
                    dt_ = smp.tile([1, 512], f32, tag="dt",
                                   name=f"dt{hp}_{half}")
                    nc.scalar.copy(dt_[:], yt_pair[half][64:65, :])
                    nc.sync.dma_start(
                        d16[2 * hp + half:2 * hp + half + 1, :], dt_[:])
                return ytu

            # ---------- software-pipelined head-pair loop ----------
            yts = []
            qk_state = qkv_rope(0)
            for hp in range(8):
                pts = sc_block(hp, *qk_state)
                if hp < 7:
                    qk_state = qkv_rope(hp + 1)
                yts.append(av_block(hp, pts))

            # ---------- batched normalization (in place, rounds to f32r) ----
            r16 = smp.tile([16, 512], f32r, tag="r16")
            with nc.allow_low_precision(reason="f32r recip for PE broadcast"):
                nc.vector.reciprocal(r16[:], d16[:])
            for hp in range(8):
                prb = psmm.tile([128, 512], f32, tag="mm", name=f"prb{hp}")
                nc.tensor.matmul(prb[:], trsel[:, hp * 128:(hp + 1) * 128],
                                 r16[:], start=True, stop=True)
                nc.vector.tensor_mul(yts[hp][0:64, :], yts[hp][0:64, :],
                                     prb[0:64, :])
                nc.vector.tensor_mul(yts[hp][64:128, :], yts[hp][64:128, :],
                                     prb[64:128, :])

            # ---------- projection (transposed output) ----------
            for cc in range(8):
                wp_sl = wsl.tile([128, 1024], f32r, tag="wslab",
                                 name=f"wp{cc}")
                nc.sync.dma_start(wp_sl[:], wps[cc * 128:(cc + 1) * 128, :])
                po = psmm.tile([128, 512], f32, tag="mm", name=f"po{cc}")
                for hp in range(8):
                    nc.tensor.matmul(
                        po[:], wp_sl[:, hp * 128:(hp + 1) * 128], yts[hp][:],
                        start=(hp == 0), stop=(hp == 7),
                    )
                osb = big.tile([128, 512], f32, tag="big", name=f"osb{cc}")
                nc.scalar.copy(osb[:], po[:])
                nc.sync.dma_start(outT[cc * 128:(cc + 1) * 128, :], osb[:])

    nc.compile()
    return nc


def _host_inputs(x, w_attn, w_proj):
    """Build the 8 per-core input maps."""
    inv_freq = 1.0 / (10000.0 ** (np.arange(0, HD, 2, dtype=np.float32) / HD))
    iff = np.concatenate([inv_freq, inv_freq])  # [64]

    def cos_sin(pos):
        ang = pos[None, :].astype(np.float32) * iff[:, None]
        c = np.concatenate([np.cos(ang), np.cos(ang)], 0).astype(np.float32)
        s = np.concatenate([np.sin(ang), np.sin(ang)], 0).astype(np.float32)
        return np.ascontiguousarray(c), np.ascontiguousarray(s)

    P2 = np.zeros((128, 128), np.float32)
    for blk in range(2):
        o = blk * 64
        for d in range(32):
            P2[o + d + 32, o + d] = -1.0
            P2[o + d, o + d + 32] = 1.0

    rsel = np.zeros((16, C), np.float32)
    for h in range(16):
        hp, half = h // 2, h % 2
        rsel[h, hp * 128 + half * 64: hp * 128 + half * 64 + 64] = 1.0
    ones16 = np.ones((128, 16), np.float32)

    def shuffle_lhsT(w):
        # rows (kc*128 + c_lo), cols (hp*128 + d) ->
        # rows (hp*128 + c_lo), cols (kc*128 + d)
        return np.ascontiguousarray(
            w.reshape(8, 128, 8, 128).transpose(2, 1, 0, 3).reshape(C, C)
        )

    wq = shuffle_lhsT(w_attn[:, 0:C])
    wk = shuffle_lhsT(w_attn[:, C:2 * C])
    wvm = np.ascontiguousarray(w_attn[:, 2 * C:3 * C])
    wp = shuffle_lhsT(w_proj)

    in_maps = []
    for core in range(NCORES):
        b, j = core // 4, core % 4
        q0 = j * CH
        kv_gk = np.full(KV, -1, np.int64)
        kv_gk[0:512] = q0 + np.arange(CH)
        halo = q0 - 256 + np.arange(256)
        kv_gk[512:768] = np.where(halo >= 0, halo, -1)
        kv_gk[768:772] = np.arange(4)

        xTc = np.zeros((C, KV), np.float32)
        valid = kv_gk >= 0
        xTc[:, valid] = x[b, kv_gk[valid]].T

        cq, sq = cos_sin(q0 + np.arange(CH))
        ck, sk = cos_sin(np.maximum(kv_gk, 0))

        gq = q0 + np.arange(CH)
        mask = np.zeros((128, MTOT), np.float32)
        for c in range(7):
            rows = c * 128 + np.arange(128)
            gk = kv_gk[rows]
            qw = gq[OFF_C[c]:OFF_C[c] + W_C[c]]
            real = (rows < 772) & (gk >= 0)
            g = np.where(real, gk, 0)[:, None]
            qq = qw[None, :]
            is_sink = ((rows >= 768) & (rows < 772))[:, None]
            allow = np.where(
                is_sink,
                (g <= qq) & (qq - g >= WIN),
                (g <= qq) & (qq - g < WIN),
            )
            allow &= real[:, None]
            mask[:, MOFF[c]:MOFF[c] + W_C[c]] = allow.astype(np.float32)

        in_maps.append({
            "xT": xTc, "wqs": wq, "wks": wk, "wv": wvm, "wps": wp,
            "cos_q": cq, "sin_q": sq, "cos_k": ck, "sin_k": sk,
            "masks": mask, "p2": P2, "rsel": rsel, "ones": ones16,
        })
    return in_maps


def kernel(x, w_attn, w_proj):
    from concourse import bass_utils

    x = np.asarray(x, np.float32)
    w_attn = np.asarray(w_attn, np.float32)
    w_proj = np.asarray(w_proj, np.float32)

    if "nc" not in _cache:
        _cache["nc"] = _build_nc()
    nc = _cache["nc"]

    in_maps = _host_inputs(x, w_attn, w_proj)
    res = bass_utils.run_bass_kernel_spmd(nc, in_maps, list(range(NCORES)),
                                          **_cache.get("run_kwargs", {}))
    _cache["last_result"] = res

    y = np.zeros((B, T, C), np.float32)
    for core in range(NCORES):
        b, j = core // 4, core % 4
        y[b, j * CH:(j + 1) * CH, :] = res.results[core]["outT"].T
    return y



# revision 12
# speedup vs baseline: 1.3370x; 1.3370x over previous
"""Trainium2 Bass kernel for CausalSelfAttention (sliding window + sink).

Sharding: 8 cores = (batch 2) x (head-group 4). Each core computes Q/K/V
for its 4 heads over the full T=2048 sequence (no halo recompute), runs
banded attention (key chunks of 128, per-512-query-block windows, sink
keys via a shared psum bank), normalizes with a ones-column denominator,
projects through its 256 rows of w_proj, and returns a transposed partial
output [C, T] in bf16. The host sums the 4 per-batch partials.

All matmuls are bf16 (fp32 psum accumulation); probabilities bf16.
"""

import numpy as np
import ml_dtypes

B, T, C, NH, HD = 2, 2048, 1024, 16, 64
WIN, SINK = 256, 4
G, HPG, DG = 4, 4, 256   # head groups, heads/group, dims/group
NT, TCH = 4, 512         # t-chunks
NCORES = 8

BF = ml_dtypes.bfloat16

_cache = {}


def _sc_layout(qc):
    """Score-matmul layout for query block qc.

    Returns (sc_list, av_list). sc entries: (kc, qoff, w, bank, boff).
    av entries: (kc, qoff, w, bank, boff, start). AV must cover all
    elements with start=True before any accumulate touches them.
    """
    if qc == 0:
        sc = [
            (0, 0, 512, 0, 0),
            (1, 128, 384, 1, 0),
            (3, 384, 128, 1, 384),
            (2, 256, 256, 2, 0),
        ]
        av = [
            (0, 0, 512, 0, 0, True),
            (1, 128, 384, 1, 0, False),
            (2, 256, 256, 2, 0, False),
            (3, 384, 128, 1, 384, False),
        ]
    else:
        k0 = 4 * qc
        sc = [
            (k0, 0, 384, 0, 0),
            (k0 - 2, 0, 128, 0, 384),
            (k0 + 1, 128, 384, 1, 0),
            (k0 + 3, 384, 128, 1, 384),
            (k0 - 1, 0, 256, 2, 0),
            (k0 + 2, 256, 256, 2, 256),
        ]
        av = [
            (k0, 0, 384, 0, 0, True),
            (k0 + 3, 384, 128, 1, 384, True),
            (k0 - 2, 0, 128, 0, 384, False),
            (k0 - 1, 0, 256, 2, 0, False),
            (k0 + 1, 128, 384, 1, 0, False),
            (k0 + 2, 256, 256, 2, 256, False),
        ]
    return sc, av


EXPW = {0: [512, 512, 256], 1: [512, 512, 512]}  # used width per score bank


def _build_nc():
    import concourse.bacc as bacc
    import concourse.mybir as mybir
    import concourse.tile as tile

    f32 = mybir.dt.float32
    bf16 = mybir.dt.bfloat16
    AF = mybir.ActivationFunctionType

    nc = bacc.Bacc("TRN2", target_bir_lowering=False, debug=False,
                   num_devices=NCORES)

    xT = nc.dram_tensor("xT", [C, T], bf16, kind="ExternalInput").ap()
    wq = nc.dram_tensor("wq", [128, 2048], bf16, kind="ExternalInput").ap()
    wk = nc.dram_tensor("wk", [128, 2048], bf16, kind="ExternalInput").ap()
    wv = nc.dram_tensor("wv", [128, 2048], bf16, kind="ExternalInput").ap()
    wp = nc.dram_tensor("wp", [128, 2048], bf16, kind="ExternalInput").ap()
    cosd = nc.dram_tensor("cos", [128, T], bf16, kind="ExternalInput").ap()
    sind = nc.dram_tensor("sin", [128, T], bf16, kind="ExternalInput").ap()
    p2d = nc.dram_tensor("p2", [128, 128], bf16, kind="ExternalInput").ap()
    mask1d = nc.dram_tensor("mask1", [128, 1536], bf16,
                            kind="ExternalInput").ap()
    mask0d = nc.dram_tensor("mask0", [128, 1280], bf16,
                            kind="ExternalInput").ap()
    rseld = nc.dram_tensor("rsel", [4, 256], bf16, kind="ExternalInput").ap()
    outT = nc.dram_tensor("outT", [C, T], bf16, kind="ExternalOutput").ap()

    with tile.TileContext(nc) as tc:
        with (
            tc.tile_pool(name="pers", bufs=1) as pers,
            tc.tile_pool(name="sb", bufs=1) as sb,
            tc.tile_pool(name="wk2", bufs=2) as wk2,
            tc.tile_pool(name="pr", bufs=2) as prp,
            tc.tile_pool(name="ps", bufs=1, space="PSUM") as ps,
        ):
            # ---------------- persistent loads ----------------
            wq_t = pers.tile([128, 2048], bf16, tag="wq")
            nc.sync.dma_start(wq_t[:], wq[:])
            xts = []
            for i in range(8):
                t = pers.tile([128, T], bf16, tag=f"xt{i}", name=f"xt{i}")
                nc.sync.dma_start(t[:, 0:1024], xT[i * 128:(i + 1) * 128,
                                                   0:1024])
                xts.append(t)
            wk_t = pers.tile([128, 2048], bf16, tag="wk")
            nc.sync.dma_start(wk_t[:], wk[:])
            wv_t = pers.tile([128, 2048], bf16, tag="wv")
            nc.sync.dma_start(wv_t[:], wv[:])
            tcos = pers.tile([128, T], bf16, tag="cos")
            nc.sync.dma_start(tcos[:], cosd[:])
            tsin = pers.tile([128, T], bf16, tag="sin")
            nc.sync.dma_start(tsin[:], sind[:])
            tp2 = pers.tile([128, 128], bf16, tag="p2")
            nc.sync.dma_start(tp2[:], p2d[:])
            for i in range(8):
                nc.sync.dma_start(xts[i][:, 1024:2048],
                                  xT[i * 128:(i + 1) * 128, 1024:2048])
            tmask1 = pers.tile([128, 1536], bf16, tag="mask1")
            nc.sync.dma_start(tmask1[:], mask1d[:])
            tmask0 = pers.tile([128, 1280], bf16, tag="mask0")
            nc.sync.dma_start(tmask0[:], mask0d[:])
            trsel = pers.tile([4, 256], bf16, tag="rsel")
            nc.sync.dma_start(trsel[:], rseld[:])
            wp_t = pers.tile([128, 2048], bf16, tag="wp")
            nc.sync.dma_start(wp_t[:], wp[:])

            # ---------------- QKV + RoPE ----------------
            # qT/kT in [d, T] layout: tile dt holds heads 2dt, 2dt+1.
            qT = [sb.tile([128, T], bf16, tag=f"qT{d}", name=f"qT{d}")
                  for d in range(2)]
            kT = [sb.tile([128, T], bf16, tag=f"kT{d}", name=f"kT{d}")
                  for d in range(2)]
            qraw = [sb.tile([128, T], bf16, tag=f"qraw{d}", name=f"qraw{d}")
                    for d in range(2)]
            kraw = [sb.tile([128, T], bf16, tag=f"kraw{d}", name=f"kraw{d}")
                    for d in range(2)]

            def proj_stage(w_t, dt, dst_raw, nm):
                # dst_raw[:, tc] = (w[:, dt].T @ x)[:, tc] for 4 t-chunks
                for tci in range(NT):
                    acc = ps.tile([128, 512], f32, tag=f"m{tci % 2}",
                                  name=f"acc_{nm}{dt}_{tci}")
                    for kc in range(8):
                        nc.tensor.matmul(
                            acc[:], w_t[:, (kc * 2 + dt) * 128:
                                        (kc * 2 + dt + 1) * 128],
                            xts[kc][:, tci * 512:(tci + 1) * 512],
                            start=(kc == 0), stop=(kc == 7),
                        )
                    nc.vector.tensor_copy(
                        dst_raw[:, tci * 512:(tci + 1) * 512], acc[:])

            def rope_stage(raw, dst, nm):
                for tci in range(NT):
                    sl = slice(tci * 512, (tci + 1) * 512)
                    prot = ps.tile([128, 512], f32, tag=f"m{tci % 2}",
                                   name=f"rot_{nm}_{tci}")
                    nc.tensor.matmul(prot[:], tp2[:], raw[:, sl],
                                     start=True, stop=True)
                    tmp = wk2.tile([128, 512], bf16, tag="ropetmp",
                                   name=f"rt_{nm}_{tci}")
                    nc.vector.tensor_mul(tmp[:], prot[:], tsin[:, sl])
                    nc.gpsimd.tensor_mul(dst[:, sl], raw[:, sl], tcos[:, sl])
                    nc.vector.tensor_add(dst[:, sl], dst[:, sl], tmp[:])

            for dt in range(2):
                proj_stage(wq_t, dt, qraw[dt], "q")
                rope_stage(qraw[dt], qT[dt], f"q{dt}")
            for dt in range(2):
                proj_stage(wk_t, dt, kraw[dt], "k")
                rope_stage(kraw[dt], kT[dt], f"k{dt}")

            # V in [keys, d] layout with ones column: vsb[kt] [128, 4*65]
            vsb = []
            for kt in range(16):
                pv = ps.tile([128, 512], f32, tag=f"m{kt % 2}",
                             name=f"pv{kt}")
                for kc in range(8):
                    nc.tensor.matmul(
                        pv[:, 0:256], xts[kc][:, kt * 128:(kt + 1) * 128],
                        wv_t[:, kc * 256:(kc + 1) * 256],
                        start=(kc == 0), stop=(kc == 7),
                    )
                vt = sb.tile([128, 260], bf16, tag=f"v{kt}", name=f"v{kt}")
                vr = vt.rearrange("p (h e) -> p h e", e=65)
                nc.scalar.copy(
                    vr[:, :, 0:64],
                    pv[:, 0:256].rearrange("p (h e) -> p h e", e=64))
                nc.gpsimd.memset(vr[:, :, 64:65], 1.0)
                vsb.append(vt)

            # v_sink tiles: pass p covers heads (2p, 2p+1) at partition
            # strips 0 and 64 (matmul out base must be 0/32/64).
            v_sink = []
            for p in range(2):
                vs = sb.tile([128, 65], bf16, tag=f"vsink{p}",
                             name=f"vsink{p}")
                nc.gpsimd.memset(vs[:], 0.0)
                for e in range(2):
                    h = 2 * p + e
                    nc.scalar.copy(vs[64 * e:64 * e + 4, 0:65],
                                   vsb[0][0:4, 65 * h:65 * h + 65])
                v_sink.append(vs)

            # ---------------- attention ----------------
            ytu_raw = [sb.tile([128, T], f32, tag=f"ytr{d}", name=f"ytr{d}")
                       for d in range(2)]
            dn4 = sb.tile([4, T], f32, tag="dn4")

            for qc in range(4):
                snkpr = None
                if qc >= 1:
                    # pass p: heads 2p, 2p+1 at psum strips 0 / 64
                    snkpr = []
                    for p in range(2):
                        snk = ps.tile([128, 512], f32, tag="sk",
                                      name=f"snk{qc}_{p}")
                        for e in range(2):
                            h = 2 * p + e
                            dtile = h // 2
                            dsl = slice((h % 2) * 64, (h % 2) * 64 + 64)
                            nc.tensor.matmul(
                                snk[64 * e:64 * e + 4, :],
                                kT[dtile][dsl, 0:4],
                                qT[dtile][dsl, qc * 512:(qc + 1) * 512],
                                start=True, stop=True,
                            )
                        sp = prp.tile([128, 512], bf16, tag=f"snkpr{p}",
                                      name=f"snkpr{qc}_{p}")
                        nc.scalar.activation(sp[0:68, :], snk[0:68, :],
                                             AF.Exp, scale=0.125)
                        snkpr.append(sp)

                sc_l, av_l = _sc_layout(qc)
                tmask = tmask0 if qc == 0 else tmask1
                moff = [0, 512, 1024] if qc == 0 else [0, 512, 1024]
                for h in range(4):
                    dtile, dsl = h // 2, slice((h % 2) * 64,
                                               (h % 2) * 64 + 64)
                    qsl = slice(qc * 512, (qc + 1) * 512)
                    scb = [ps.tile([128, 512], f32, tag=f"s{b}",
                                   name=f"sc{qc}_{h}_{b}") for b in range(3)]
                    for (kc, qoff, w, bank, boff) in sc_l:
                        nc.tensor.matmul(
                            scb[bank][:, boff:boff + w],
                            kT[dtile][dsl, kc * 128:(kc + 1) * 128],
                            qT[dtile][dsl, qc * 512 + qoff:
                                      qc * 512 + qoff + w],
                            start=True, stop=True,
                        )
                    prb_t = [prp.tile([128, 512], bf16, tag=f"pr{b}",
                                      name=f"pr{qc}_{h}_{b}")
                             for b in range(3)]
                    for b in range(3):
                        wb = EXPW[min(qc, 1)][b]
                        nc.scalar.activation(prb_t[b][:, 0:wb],
                                             scb[b][:, 0:wb],
                                             AF.Exp, scale=0.125)
                        nc.gpsimd.tensor_mul(
                            prb_t[b][:, 0:wb], prb_t[b][:, 0:wb],
                            tmask[:, moff[b]:moff[b] + wb])

                    yt = ps.tile([128, 512], f32, tag=f"y{h % 2}",
                                 name=f"yt{qc}_{h}")
                    nmm = len(av_l)
                    if qc >= 1:
                        # sink covers the full 512 block: open the group
                        p, e = h // 2, h % 2
                        nc.tensor.matmul(
                            yt[0:65, :],
                            v_sink[p][64 * e:64 * e + 4, 0:65],
                            snkpr[p][64 * e:64 * e + 4, :],
                            start=True, stop=False,
                        )
                    for i, (kc, qoff, w, bank, boff, st) in enumerate(av_l):
                        nc.tensor.matmul(
                            yt[0:65, qoff:qoff + w],
                            vsb[kc][:, 65 * h:65 * h + 65],
                            prb_t[bank][:, boff:boff + w],
                            start=(st and qc == 0),
                            stop=(i == nmm - 1),
                        )
                    # yt -> sbuf: rows 0..63 to ytu_raw, row 64 = denom
                    nc.scalar.copy(
                        ytu_raw[dtile][(h % 2) * 64:(h % 2) * 64 + 64, qsl],
                        yt[0:64, :])
                    dstg = wk2.tile([1, 512], f32, tag="dstg",
                                    name=f"dstg{qc}_{h}")
                    nc.scalar.copy(dstg[:], yt[64:65, :])
                    nc.sync.dma_start(dn4[h:h + 1, qsl], dstg[:])

            # ---------------- normalize + project ----------------
            r4r = sb.tile([4, T], bf16, tag="r4r")
            with nc.allow_low_precision(reason="bf16 recip for PE broadcast"):
                nc.vector.reciprocal(r4r[:], dn4[:])
            ytu = [sb.tile([128, T], bf16, tag=f"ytu{d}", name=f"ytu{d}")
                   for d in range(2)]
            for dt in range(2):
                for tci in range(NT):
                    sl = slice(tci * 512, (tci + 1) * 512)
                    prb = ps.tile([128, 512], f32, tag=f"m{tci % 2}",
                                  name=f"prb{dt}_{tci}")
                    nc.tensor.matmul(prb[:], trsel[:, dt * 128:(dt + 1) * 128],
                                     r4r[:, sl], start=True, stop=True)
                    nc.vector.tensor_mul(ytu[dt][:, sl], ytu_raw[dt][:, sl],
                                         prb[:])

            for cc in range(8):
                osb = prp.tile([128, T], bf16, tag="osb", name=f"osb{cc}")
                for tci in range(NT):
                    po = ps.tile([128, 512], f32, tag=f"m{tci % 2}",
                                 name=f"po{cc}_{tci}")
                    for dt in range(2):
                        nc.tensor.matmul(
                            po[:], wp_t[:, (dt * 8 + cc) * 128:
                                        (dt * 8 + cc + 1) * 128],
                            ytu[dt][:, tci * 512:(tci + 1) * 512],
                            start=(dt == 0), stop=(dt == 1),
                        )
                    if tci % 2 == 0:
                        nc.vector.tensor_copy(
                            osb[:, tci * 512:(tci + 1) * 512], po[:])
                    else:
                        nc.scalar.copy(
                            osb[:, tci * 512:(tci + 1) * 512], po[:])
                nc.gpsimd.dma_start(outT[cc * 128:(cc + 1) * 128, :], osb[:])

    nc.compile()
    return nc


def _host_inputs(x, w_attn, w_proj):
    """Build the 8 per-core input maps (core = 4*b + g)."""
    inv_freq = 1.0 / (10000.0 ** (np.arange(0, HD, 2, dtype=np.float32) / HD))
    iff = np.concatenate([inv_freq, inv_freq])  # [64]
    tpos = np.arange(T, dtype=np.float32)
    ang = tpos[None, :] * iff[:, None]            # [64, T]
    cos1 = np.cos(ang).astype(np.float32)
    sin1 = np.sin(ang).astype(np.float32)
    cos_t = np.concatenate([cos1, cos1], 0)       # [128, T]
    sin_t = np.concatenate([sin1, sin1], 0)

    P2 = np.zeros((128, 128), np.float32)
    for blk in range(2):
        o = blk * 64
        for d in range(32):
            P2[o + d + 32, o + d] = -1.0
            P2[o + d, o + d + 32] = 1.0

    k = np.arange(128)[:, None]
    j = np.arange(128)[None, :]
    diag = (j >= k).astype(np.float32)
    tail = (j < k).astype(np.float32)
    ones = np.ones((128, 128), np.float32)

    jj = np.arange(512)[None, :]
    kc0sp = ((jj >= k) & ((jj - k < WIN) | (k < SINK))).astype(np.float32)

    gen384 = np.concatenate([diag, ones, tail], 1)          # [128, 384]
    mask1 = np.concatenate(
        [gen384, tail, gen384, diag, ones, tail, diag, ones], 1)
    assert mask1.shape[1] == 1536
    mask0 = np.concatenate([kc0sp, gen384, diag, diag, ones], 1)
    assert mask0.shape[1] == 1280

    rsel = np.zeros((4, 256), np.float32)
    for dt in range(2):
        for b_ in range(2):
            rsel[2 * dt + b_, dt * 128 + 64 * b_:
                 dt * 128 + 64 * b_ + 64] = 1.0

    def pack_lhsT(w):
        # w [1024, 256] -> [128, (kc, dt, 128)=2048]
        return np.ascontiguousarray(
            w.reshape(8, 128, 2, 128).transpose(1, 0, 2, 3).reshape(128, 2048))

    def pack_rhs(w):
        # w [1024, 256] -> [128, (kc, 256)=2048]
        return np.ascontiguousarray(
            w.reshape(8, 128, 256).transpose(1, 0, 2).reshape(128, 2048))

    def pack_wp(w):
        # w [256, 1024] -> [128, (dt, cc, 128)=2048]
        return np.ascontiguousarray(
            w.reshape(2, 128, 8, 128).transpose(1, 0, 2, 3).reshape(128, 2048))

    bfc = lambda a: np.ascontiguousarray(a.astype(BF))

    xTb = [bfc(x[b].T) for b in range(B)]
    in_maps = []
    for core in range(NCORES):
        b, g = core // 4, core % 4
        csl = slice(g * DG, (g + 1) * DG)
        in_maps.append({
            "xT": xTb[b],
            "wq": bfc(pack_lhsT(w_attn[:, 0 * C:1 * C][:, csl])),
            "wk": bfc(pack_lhsT(w_attn[:, 1 * C:2 * C][:, csl])),
            "wv": bfc(pack_rhs(w_attn[:, 2 * C:3 * C][:, csl])),
            "wp": bfc(pack_wp(w_proj[csl, :])),
            "cos": bfc(cos_t), "sin": bfc(sin_t), "p2": bfc(P2),
            "mask1": bfc(mask1), "mask0": bfc(mask0), "rsel": bfc(rsel),
        })
    return in_maps


def kernel(x, w_attn, w_proj):
    from concourse import bass_utils

    x = np.asarray(x, np.float32)
    w_attn = np.asarray(w_attn, np.float32)
    w_proj = np.asarray(w_proj, np.float32)

    if "nc" not in _cache:
        _cache["nc"] = _build_nc()
    nc = _cache["nc"]

    in_maps = _host_inputs(x, w_attn, w_proj)
    res = bass_utils.run_bass_kernel_spmd(nc, in_maps, list(range(NCORES)),
                                          **_cache.get("run_kwargs", {}))
    _cache["last_result"] = res

    y = np.zeros((B, T, C), np.float32)
    for core in range(NCORES):
        b = core // 4
        y[b] += res.results[core]["outT"].T.astype(np.float32)
    return y


# revision 13
# speedup vs baseline: 1.6117x; 1.2055x over previous
"""Trainium2 Bass kernel for CausalSelfAttention (sliding window + sink).

Sharding: 8 cores = (batch 2) x (head-group 4). Each core computes Q/K/V
for its 4 heads over the full T=2048 sequence (no halo recompute), runs
banded attention (key chunks of 128, per-512-query-block windows, sink
keys precomputed upfront), normalizes with a ones-column denominator,
projects through its 256 rows of w_proj, and returns a transposed partial
output [C, T] in bf16. The host sums the 4 per-batch partials.

All matmuls bf16 (fp32 psum). The attention loop is software-pipelined:
scores for iteration i+1 are issued before AV of iteration i, with score
psum banks double-buffered by iteration parity (tags s0..s5, yt y0/y1).
"""

import numpy as np
import ml_dtypes

B, T, C, NH, HD = 2, 2048, 1024, 16, 64
WIN, SINK = 256, 4
G, HPG, DG = 4, 4, 256   # head groups, heads/group, dims/group
NT, TCH = 4, 512         # t-chunks
NCORES = 8

BF = ml_dtypes.bfloat16

_cache = {}


def _sc_layout(qc):
    """Score/AV layout for query block qc: (kc, qoff, w, bank, boff)."""
    if qc == 0:
        sc = [
            (0, 0, 512, 0, 0),
            (1, 128, 384, 1, 0),
            (3, 384, 128, 1, 384),
            (2, 256, 256, 2, 0),
        ]
        av = [
            (0, 0, 512, 0, 0, True),
            (1, 128, 384, 1, 0, False),
            (2, 256, 256, 2, 0, False),
            (3, 384, 128, 1, 384, False),
        ]
    else:
        k0 = 4 * qc
        sc = [
            (k0, 0, 384, 0, 0),
            (k0 - 2, 0, 128, 0, 384),
            (k0 + 1, 128, 384, 1, 0),
            (k0 + 3, 384, 128, 1, 384),
            (k0 - 1, 0, 256, 2, 0),
            (k0 + 2, 256, 256, 2, 256),
        ]
        av = [
            (k0, 0, 384, 0, 0, False),
            (k0 + 3, 384, 128, 1, 384, False),
            (k0 - 2, 0, 128, 0, 384, False),
            (k0 - 1, 0, 256, 2, 0, False),
            (k0 + 1, 128, 384, 1, 0, False),
            (k0 + 2, 256, 256, 2, 256, False),
        ]
    return sc, av


EXPW = {0: [512, 512, 256], 1: [512, 512, 512]}


def _build_nc():
    import concourse.bacc as bacc
    import concourse.mybir as mybir
    import concourse.tile as tile

    f32 = mybir.dt.float32
    bf16 = mybir.dt.bfloat16
    AF = mybir.ActivationFunctionType

    nc = bacc.Bacc("TRN2", target_bir_lowering=False, debug=False,
                   num_devices=NCORES)

    xT = nc.dram_tensor("xT", [C, T], bf16, kind="ExternalInput").ap()
    wq = nc.dram_tensor("wq", [128, 2048], bf16, kind="ExternalInput").ap()
    wk = nc.dram_tensor("wk", [128, 2048], bf16, kind="ExternalInput").ap()
    wv = nc.dram_tensor("wv", [128, 2048], bf16, kind="ExternalInput").ap()
    wp = nc.dram_tensor("wp", [128, 2048], bf16, kind="ExternalInput").ap()
    cosd = nc.dram_tensor("cos", [128, T], bf16, kind="ExternalInput").ap()
    sind = nc.dram_tensor("sin", [128, T], bf16, kind="ExternalInput").ap()
    p2d = nc.dram_tensor("p2", [128, 128], bf16, kind="ExternalInput").ap()
    mask1d = nc.dram_tensor("mask1", [128, 1536], bf16,
                            kind="ExternalInput").ap()
    mask0d = nc.dram_tensor("mask0", [128, 1280], bf16,
                            kind="ExternalInput").ap()
    rseld = nc.dram_tensor("rsel", [16, 1024], bf16,
                           kind="ExternalInput").ap()
    outT = nc.dram_tensor("outT", [C, T], bf16, kind="ExternalOutput").ap()

    with tile.TileContext(nc) as tc:
        with (
            tc.tile_pool(name="pers", bufs=1) as pers,
            tc.tile_pool(name="sb", bufs=1) as sb,
            tc.tile_pool(name="wk2", bufs=2) as wk2,
            tc.tile_pool(name="pr", bufs=2) as prp,
            tc.tile_pool(name="ps", bufs=1, space="PSUM") as ps,
        ):
            # ---------------- persistent loads ----------------
            wq_t = pers.tile([128, 2048], bf16, tag="wq")
            nc.sync.dma_start(wq_t[:], wq[:])
            xts = []
            for i in range(8):
                t = pers.tile([128, T], bf16, tag=f"xt{i}", name=f"xt{i}")
                nc.sync.dma_start(t[:, 0:1024], xT[i * 128:(i + 1) * 128,
                                                   0:1024])
                xts.append(t)
            wk_t = pers.tile([128, 2048], bf16, tag="wk")
            nc.sync.dma_start(wk_t[:], wk[:])
            wv_t = pers.tile([128, 2048], bf16, tag="wv")
            nc.sync.dma_start(wv_t[:], wv[:])
            tcos = pers.tile([128, T], bf16, tag="cos")
            nc.sync.dma_start(tcos[:], cosd[:])
            tsin = pers.tile([128, T], bf16, tag="sin")
            nc.sync.dma_start(tsin[:], sind[:])
            tp2 = pers.tile([128, 128], bf16, tag="p2")
            nc.sync.dma_start(tp2[:], p2d[:])
            for i in range(8):
                nc.sync.dma_start(xts[i][:, 1024:2048],
                                  xT[i * 128:(i + 1) * 128, 1024:2048])
            tmask1 = pers.tile([128, 1536], bf16, tag="mask1")
            nc.sync.dma_start(tmask1[:], mask1d[:])
            tmask0 = pers.tile([128, 1280], bf16, tag="mask0")
            nc.sync.dma_start(tmask0[:], mask0d[:])
            trsel = pers.tile([16, 1024], bf16, tag="rsel")
            nc.sync.dma_start(trsel[:], rseld[:])
            wp_t = pers.tile([128, 2048], bf16, tag="wp")
            nc.sync.dma_start(wp_t[:], wp[:])

            # ---------------- QKV + RoPE ----------------
            qT = [sb.tile([128, T], bf16, tag=f"qT{d}", name=f"qT{d}")
                  for d in range(2)]
            kT = [sb.tile([128, T], bf16, tag=f"kT{d}", name=f"kT{d}")
                  for d in range(2)]
            qraw = [sb.tile([128, T], bf16, tag=f"qraw{d}", name=f"qraw{d}")
                    for d in range(2)]
            kraw = [sb.tile([128, T], bf16, tag=f"kraw{d}", name=f"kraw{d}")
                    for d in range(2)]

            def proj_stage(w_t, dt, dst_raw, nm):
                for tci in range(NT):
                    acc = ps.tile([128, 512], f32, tag=f"s{tci}",
                                  name=f"acc_{nm}{dt}_{tci}")
                    for kc in range(8):
                        nc.tensor.matmul(
                            acc[:], w_t[:, (kc * 2 + dt) * 128:
                                        (kc * 2 + dt + 1) * 128],
                            xts[kc][:, tci * 512:(tci + 1) * 512],
                            start=(kc == 0), stop=(kc == 7),
                        )
                    nc.vector.tensor_copy(
                        dst_raw[:, tci * 512:(tci + 1) * 512], acc[:])

            def rope_stage(raw, dst, nm):
                for tci in range(NT):
                    sl = slice(tci * 512, (tci + 1) * 512)
                    prot = ps.tile([128, 512], f32, tag=f"s{4 + tci % 2}",
                                   name=f"rot_{nm}_{tci}")
                    nc.tensor.matmul(prot[:], tp2[:], raw[:, sl],
                                     start=True, stop=True)
                    tmp = wk2.tile([128, 512], bf16, tag="ropetmp",
                                   name=f"rt_{nm}_{tci}")
                    nc.vector.tensor_mul(tmp[:], prot[:], tsin[:, sl])
                    nc.gpsimd.tensor_mul(dst[:, sl], raw[:, sl], tcos[:, sl])
                    nc.gpsimd.tensor_add(dst[:, sl], dst[:, sl], tmp[:])

            for dt in range(2):
                proj_stage(wq_t, dt, qraw[dt], "q")
                rope_stage(qraw[dt], qT[dt], f"q{dt}")
            for dt in range(2):
                proj_stage(wk_t, dt, kraw[dt], "k")
                rope_stage(kraw[dt], kT[dt], f"k{dt}")

            # V in [keys, d] layout with ones column
            vsb = []
            for kt in range(16):
                pv = ps.tile([128, 512], f32, tag=f"s{kt % 4}",
                             name=f"pv{kt}")
                for kc in range(8):
                    nc.tensor.matmul(
                        pv[:, 0:256], xts[kc][:, kt * 128:(kt + 1) * 128],
                        wv_t[:, kc * 256:(kc + 1) * 256],
                        start=(kc == 0), stop=(kc == 7),
                    )
                vt = sb.tile([128, 260], bf16, tag=f"v{kt}", name=f"v{kt}")
                vr = vt.rearrange("p (h e) -> p h e", e=65)
                nc.vector.tensor_copy(
                    vr[:, :, 0:64],
                    pv[:, 0:256].rearrange("p (h e) -> p h e", e=64))
                nc.gpsimd.memset(vr[:, :, 64:65], 1.0)
                vsb.append(vt)

            v_sink = []
            for p in range(2):
                vs = sb.tile([128, 65], bf16, tag=f"vsink{p}",
                             name=f"vsink{p}")
                nc.gpsimd.memset(vs[:], 0.0)
                for e in range(2):
                    h = 2 * p + e
                    nc.scalar.copy(vs[64 * e:64 * e + 4, 0:65],
                                   vsb[0][0:4, 65 * h:65 * h + 65])
                v_sink.append(vs)

            # ---------------- sink scores upfront ----------------
            # snkpr[qc][p]: heads (2p, 2p+1) at strips 0/64, exp'd probs
            snkpr = {}
            for qc in range(1, 4):
                for p in range(2):
                    snk = ps.tile([128, 512], f32,
                                  tag=f"s{(2 * qc + p) % 6}",
                                  name=f"snk{qc}_{p}")
                    for e in range(2):
                        h = 2 * p + e
                        dtile = h // 2
                        dsl = slice((h % 2) * 64, (h % 2) * 64 + 64)
                        nc.tensor.matmul(
                            snk[64 * e:64 * e + 4, :],
                            kT[dtile][dsl, 0:4],
                            qT[dtile][dsl, qc * 512:(qc + 1) * 512],
                            start=True, stop=True,
                        )
                    sp = sb.tile([128, 512], bf16, tag=f"snkpr{qc}_{p}",
                                 name=f"snkpr{qc}_{p}")
                    nc.scalar.activation(sp[0:68, :], snk[0:68, :],
                                         AF.Exp, scale=0.125)
                    snkpr[(qc, p)] = sp

            # ---------------- pipelined attention ----------------
            stg = {}   # (h, qc) -> [65, 512] f32 sbuf (rows 0-63 y, 64 den)
            dn16 = sb.tile([16, 512], f32, tag="dn16")

            def emit_scores(i, qc, h):
                dtile = h // 2
                dsl = slice((h % 2) * 64, (h % 2) * 64 + 64)
                sc_l, _ = _sc_layout(qc)
                par = 3 * (i % 2)
                scb = [ps.tile([128, 512], f32, tag=f"s{par + b}",
                               name=f"sc{qc}_{h}_{b}") for b in range(3)]
                for (kc, qoff, w, bank, boff) in sc_l:
                    nc.tensor.matmul(
                        scb[bank][:, boff:boff + w],
                        kT[dtile][dsl, kc * 128:(kc + 1) * 128],
                        qT[dtile][dsl, qc * 512 + qoff:qc * 512 + qoff + w],
                        start=True, stop=True,
                    )
                probs = prp.tile([128, 1536], bf16, tag="pr",
                                 name=f"pr{qc}_{h}")
                for b in range(3):
                    wb = EXPW[min(qc, 1)][b]
                    nc.scalar.activation(probs[:, 512 * b:512 * b + wb],
                                         scb[b][:, 0:wb],
                                         AF.Exp, scale=0.125)
                tm, tw = (tmask0, 1280) if qc == 0 else (tmask1, 1536)
                nc.vector.tensor_mul(probs[:, 0:tw], probs[:, 0:tw],
                                     tm[:, 0:tw])
                return probs

            def emit_av(i, qc, h, probs):
                _, av_l = _sc_layout(qc)
                yt = ps.tile([128, 512], f32, tag=f"y{i % 2}",
                             name=f"yt{qc}_{h}")
                if qc >= 1:
                    p, e = h // 2, h % 2
                    nc.tensor.matmul(
                        yt[0:65, :],
                        v_sink[p][64 * e:64 * e + 4, 0:65],
                        snkpr[(qc, p)][64 * e:64 * e + 4, :],
                        start=True, stop=False,
                    )
                nmm = len(av_l)
                for ii, (kc, qoff, w, bank, boff, st) in enumerate(av_l):
                    nc.tensor.matmul(
                        yt[0:65, qoff:qoff + w],
                        vsb[kc][:, 65 * h:65 * h + 65],
                        probs[:, 512 * bank + boff:512 * bank + boff + w],
                        start=st, stop=(ii == nmm - 1),
                    )
                st65 = sb.tile([65, 512], f32, tag=f"stg{qc}_{h}",
                               name=f"stg{qc}_{h}")
                nc.scalar.copy(st65[:], yt[0:65, :])
                nc.sync.dma_start(dn16[4 * h + qc:4 * h + qc + 1, :],
                                  st65[64:65, :])
                stg[(h, qc)] = st65

            iters = [(qc, h) for qc in range(4) for h in range(4)]
            pend = None
            for i, (qc, h) in enumerate(iters):
                probs = emit_scores(i, qc, h)
                if pend is not None:
                    emit_av(*pend)
                pend = (i, qc, h, probs)
            emit_av(*pend)

            # ---------------- normalize + project ----------------
            r16 = sb.tile([16, 512], bf16, tag="r16")
            with nc.allow_low_precision(reason="bf16 recip for PE broadcast"):
                nc.vector.reciprocal(r16[:], dn16[:])
            ytu = [sb.tile([128, T], bf16, tag=f"ytu{d}", name=f"ytu{d}")
                   for d in range(2)]
            for h in range(4):
                for qc in range(4):
                    r = 4 * h + qc
                    dtile = h // 2
                    prb = ps.tile([128, 512], f32, tag=f"s{r % 4}",
                                  name=f"prb{r}")
                    nc.tensor.matmul(prb[0:64, :],
                                     trsel[:, r * 64:(r + 1) * 64],
                                     r16[:], start=True, stop=True)
                    nc.vector.tensor_mul(
                        ytu[dtile][(h % 2) * 64:(h % 2) * 64 + 64,
                                   qc * 512:(qc + 1) * 512],
                        stg[(h, qc)][0:64, :], prb[0:64, :])

            for cc in range(8):
                osb = prp.tile([128, T], bf16, tag="osb", name=f"osb{cc}")
                for tci in range(NT):
                    po = ps.tile([128, 512], f32, tag=f"s{tci % 4}",
                                 name=f"po{cc}_{tci}")
                    for dt in range(2):
                        nc.tensor.matmul(
                            po[:], wp_t[:, (dt * 8 + cc) * 128:
                                        (dt * 8 + cc + 1) * 128],
                            ytu[dt][:, tci * 512:(tci + 1) * 512],
                            start=(dt == 0), stop=(dt == 1),
                        )
                    if tci % 2 == 0:
                        nc.vector.tensor_copy(
                            osb[:, tci * 512:(tci + 1) * 512], po[:])
                    else:
                        nc.scalar.copy(
                            osb[:, tci * 512:(tci + 1) * 512], po[:])
                nc.gpsimd.dma_start(outT[cc * 128:(cc + 1) * 128, :], osb[:])

    nc.compile()
    return nc


def _host_inputs(x, w_attn, w_proj):
    """Build the 8 per-core input maps (core = 4*b + g)."""
    inv_freq = 1.0 / (10000.0 ** (np.arange(0, HD, 2, dtype=np.float32) / HD))
    iff = np.concatenate([inv_freq, inv_freq])  # [64]
    tpos = np.arange(T, dtype=np.float32)
    ang = tpos[None, :] * iff[:, None]            # [64, T]
    cos1 = np.cos(ang).astype(np.float32)
    sin1 = np.sin(ang).astype(np.float32)
    cos_t = np.concatenate([cos1, cos1], 0)       # [128, T]
    sin_t = np.concatenate([sin1, sin1], 0)

    P2 = np.zeros((128, 128), np.float32)
    for blk in range(2):
        o = blk * 64
        for d in range(32):
            P2[o + d + 32, o + d] = -1.0
            P2[o + d, o + d + 32] = 1.0

    k = np.arange(128)[:, None]
    j = np.arange(128)[None, :]
    diag = (j >= k).astype(np.float32)
    tail = (j < k).astype(np.float32)
    ones = np.ones((128, 128), np.float32)

    jj = np.arange(512)[None, :]
    kc0sp = ((jj >= k) & ((jj - k < WIN) | (k < SINK))).astype(np.float32)

    gen384 = np.concatenate([diag, ones, tail], 1)
    mask1 = np.concatenate(
        [gen384, tail, gen384, diag, ones, tail, diag, ones], 1)
    mask0 = np.concatenate([kc0sp, gen384, diag, diag, ones], 1)

    # rsel16: block (h, qc) at cols r*64, one-hot row r = 4h+qc
    rsel16 = np.zeros((16, 1024), np.float32)
    for r in range(16):
        rsel16[r, r * 64:(r + 1) * 64] = 1.0

    def pack_lhsT(w):
        return np.ascontiguousarray(
            w.reshape(8, 128, 2, 128).transpose(1, 0, 2, 3).reshape(128, 2048))

    def pack_rhs(w):
        return np.ascontiguousarray(
            w.reshape(8, 128, 256).transpose(1, 0, 2).reshape(128, 2048))

    def pack_wp(w):
        return np.ascontiguousarray(
            w.reshape(2, 128, 8, 128).transpose(1, 0, 2, 3).reshape(128, 2048))

    bfc = lambda a: np.ascontiguousarray(a.astype(BF))

    xTb = [bfc(x[b].T) for b in range(B)]
    in_maps = []
    for core in range(NCORES):
        b, g = core // 4, core % 4
        csl = slice(g * DG, (g + 1) * DG)
        in_maps.append({
            "xT": xTb[b],
            "wq": bfc(pack_lhsT(w_attn[:, 0 * C:1 * C][:, csl])),
            "wk": bfc(pack_lhsT(w_attn[:, 1 * C:2 * C][:, csl])),
            "wv": bfc(pack_rhs(w_attn[:, 2 * C:3 * C][:, csl])),
            "wp": bfc(pack_wp(w_proj[csl, :])),
            "cos": bfc(cos_t), "sin": bfc(sin_t), "p2": bfc(P2),
            "mask1": bfc(mask1), "mask0": bfc(mask0), "rsel": bfc(rsel16),
        })
    return in_maps


def kernel(x, w_attn, w_proj):
    from concourse import bass_utils

    x = np.asarray(x, np.float32)
    w_attn = np.asarray(w_attn, np.float32)
    w_proj = np.asarray(w_proj, np.float32)

    if "nc" not in _cache:
        _cache["nc"] = _build_nc()
    nc = _cache["nc"]

    in_maps = _host_inputs(x, w_attn, w_proj)
    res = bass_utils.run_bass_kernel_spmd(nc, in_maps, list(range(NCORES)),
                                          **_cache.get("run_kwargs", {}))
    _cache["last_result"] = res

    y = np.zeros((B, T, C), np.float32)
    for core in range(NCORES):
        b = core // 4
        y[b] += res.results[core]["outT"].T.astype(np.float32)
    return y
